# revision 9
# baseline (speedup 1.0000x reference)
"""MedianPool2d 3x3 stride-1 reflect-pad kernel for 8 TRN2 NeuronCores.

Input:  x [16, 3, 512, 512] fp32 (full). Output: same shape, lower median
of each 3x3 window after reflect pad. Computed in fp16 (tolerance 2e-2;
fp16 quantization contributes ~2e-4 norm-relative error).

Strategy:
 - Pure data parallel: 48 images (B*C) -> 6 images per core, no collectives.
 - fp16 + pair-interleaved layout: two images per plane with columns
   interleaved (I[:, 2c] = A[:, c], I[:, 2c+1] = B[:, c]). A +-1 column
   window shift is then a +-2 fp16 element offset = 4-byte aligned, so
   every tensor_tensor min/max qualifies for the DVE 2x_1P perf mode
   (16-bit dtype, step +-1, 4B-aligned -> 2 elem/cycle/lane). Vertical
   shifts are whole-slot offsets (1028 elems), also aligned.
 - Host staging: reflect pad to [514, 514], interleave pairs to [514, 1028];
   partition p holds rows [4p, 4p+6) of EVERY plane (3 blocks of 6 slots),
   so all 9 window taps are free-dim offsets and one 2D access pattern
   (outer stride = block, inner contiguous) covers all three planes ->
   the whole median is 18 maximal-size DVE instructions per iteration.
 - Median-of-9 via med3(max3(col mins), med3(col meds), min3(col maxes)),
   sequenced into 5 stat buffers + 1 output buffer to fit SBUF.
 - Output stays interleaved fp16 in DRAM; host de-interleaves + upcasts.
"""

import sys

for _p in ("/opt/trn_rl_repo", "/root/.axon_site/_ro/trn_rl_repo"):
    if _p not in sys.path:
        sys.path.append(_p)

import numpy as np

import concourse.bass as bass
import concourse.bacc as bacc
import concourse.mybir as mybir
from concourse.tile import TileContext

F16 = mybir.dt.float16
MIN = mybir.AluOpType.min
MAX = mybir.AluOpType.max

W = 512
WP2 = 1028           # interleaved padded pair-row width (2 * 514)
RPP = 4              # pair-rows per partition per plane
NSLOT = RPP + 2      # + top/bottom halo rows
FLAT2 = NSLOT * WP2  # 6168 fp16 per partition per plane block
CLEN2 = RPP * WP2    # 4112 flat stat/output length per block
NPAIR = 3            # image pairs (planes) per core


def _build_bass(loop_k=1):
    nc = bacc.Bacc("TRN2", target_bir_lowering=False)
    x_d = nc.declare_dram_parameter("x", [128, NPAIR, FLAT2], F16, isOutput=False)
    o_d = nc.declare_dram_parameter("out", [128, NPAIR, CLEN2], F16, isOutput=True)

    import contextlib
    with TileContext(nc) as tc:
        loop_cm = tc.For_i(0, loop_k, 1) if loop_k > 1 else contextlib.nullcontext()
        with loop_cm, tc.tile_pool(name="pool", bufs=1) as pool:
            xin = pool.tile([128, NPAIR, FLAT2], F16, tag="xin")
            P1 = pool.tile([128, NPAIR, CLEN2], F16, tag="p1")
            P2 = pool.tile([128, NPAIR, CLEN2], F16, tag="p2")
            S1 = pool.tile([128, NPAIR, CLEN2], F16, tag="s1")
            S2 = pool.tile([128, NPAIR, CLEN2], F16, tag="s2")
            S3 = pool.tile([128, NPAIR, CLEN2], F16, tag="s3")
            O = pool.tile([128, NPAIR, CLEN2], F16, tag="o")

            # per-block input DMAs: block b's load (next loop iteration) only
            # waits on block b's column-stage reads, so it starts ~2 blocks
            # of compute before it is needed and stays off the critical path
            for b in range(NPAIR):
                nc.sync.dma_start(out=xin[:, b], in_=x_d[:, b])

            TT = nc.vector.tensor_tensor

            def xv(off):  # xin 2D view: all blocks, inner [off, off+CLEN2)
                return xin[:, :, off : off + CLEN2]

            # column stage: vertical min/med/max per flat position
            v0, v1, v2 = xv(0), xv(WP2), xv(2 * WP2)
            TT(P1[:], v0, v1, MIN)
            TT(P2[:], v0, v1, MAX)
            TT(S1[:], P1[:], v2, MIN)        # cmin
            TT(S2[:], P2[:], v2, MAX)        # cmax
            TT(P2[:], P2[:], v2, MIN)        # t5
            TT(S3[:], P1[:], P2[:], MAX)     # cmed

            # row stage: outputs at [2, CLEN2-2) per block
            def cs(T):
                return T[:, :, 2 : CLEN2 - 2]

            def ls(T):
                return T[:, :, 0 : CLEN2 - 4]

            def rs(T):
                return T[:, :, 4:CLEN2]

            TT(cs(P1), ls(S1), rs(S1), MAX)
            TT(cs(P1), cs(P1), cs(S1), MAX)   # A = max3(cmin)
            TT(cs(P2), ls(S2), rs(S2), MIN)
            TT(cs(P2), cs(P2), cs(S2), MIN)   # C = min3(cmax)
            TT(cs(S1), ls(S3), cs(S3), MIN)
            TT(cs(S2), ls(S3), cs(S3), MAX)
            TT(cs(S2), cs(S2), rs(S3), MIN)
            TT(cs(S1), cs(S1), cs(S2), MAX)   # B = med3(cmed)
            TT(cs(S3), cs(P1), cs(S1), MIN)   # mn2
            TT(cs(P1), cs(P1), cs(S1), MAX)   # mx2
            TT(cs(P1), cs(P1), cs(P2), MIN)   # t3
            TT(cs(O), cs(S3), cs(P1), MAX)    # median

            for b in range(NPAIR):
                nc.sync.dma_start(out=o_d[:, b, 2 : CLEN2 - 2],
                                  in_=O[:, b, 2 : CLEN2 - 2])
    return nc


_NC_CACHE = None


def _get_nc():
    global _NC_CACHE
    if _NC_CACHE is None:
        nc = _build_bass()
        nc.compile()
        _NC_CACHE = nc
    return _NC_CACHE


def _stage_core(imgs):
    """imgs: [6, 512, 512] float -> staged [128, NPAIR, FLAT2] fp16: pairs
    reflect-padded, column-interleaved, 6-row sliding slots per partition."""
    imgs = np.asarray(imgs, dtype=np.float16)
    xp = np.pad(imgs, ((0, 0), (1, 1), (1, 1)), mode="reflect")  # [6, 514, 514]
    inter = np.empty((NPAIR, 514, WP2), dtype=np.float16)
    inter[:, :, 0::2] = xp[0::2]
    inter[:, :, 1::2] = xp[1::2]
    idx = np.arange(128)[:, None] * RPP + np.arange(NSLOT)[None, :]  # [128, 6]
    blocks = inter[:, idx, :]  # [NPAIR, 128, 6, 1028]
    staged = blocks.reshape(NPAIR, 128, FLAT2).transpose(1, 0, 2)
    return np.ascontiguousarray(staged)


def _unstage_core(out_d):
    """out_d: [128, NPAIR, CLEN2] fp16 -> [6, 512, 512] fp32."""
    o = out_d.transpose(1, 0, 2).reshape(NPAIR, 128, RPP, WP2)[:, :, :, 2 : 2 + 2 * W]
    o = o.reshape(NPAIR, 512, 2 * W)
    res = np.empty((6, 512, 512), dtype=np.float32)
    res[0::2] = o[:, :, 0::2].astype(np.float32)
    res[1::2] = o[:, :, 1::2].astype(np.float32)
    return res


def run(x, trace=False):
    """x: [16,3,512,512] fp32 -> (out [16,3,512,512] fp32, exec_time_ns|None)"""
    from concourse.bass_utils import run_bass_kernel_spmd

    x = np.ascontiguousarray(np.asarray(x, dtype=np.float32))
    B, C, H, Wd = x.shape
    imgs = x.reshape(8, 6, H, Wd)
    in_maps = [{"x": _stage_core(imgs[i])} for i in range(8)]
    nc = _get_nc()
    res = run_bass_kernel_spmd(nc, in_maps, list(range(8)), trace=trace)
    out = np.stack([_unstage_core(res.results[i]["out"]) for i in range(8)])
    return out.reshape(B, C, H, Wd), res.exec_time_ns


def kernel(x):
    out, _ = run(x, trace=False)
    return out
